# revision 4
# baseline (speedup 1.0000x reference)
"""Trainium2 Bass kernel for nn_Cooord_Attn (B=2,C=64,H=W=64, dual NxN attention).

Sharding: 2 cores, one batch image per core (attention is per-sample, so the
batch axis is embarrassingly parallel). The other 6 cores idle; at this size
the wall clock is dominated by host<->device transfer over the axon tunnel
(~14 MB/s), so the design minimizes wire bytes:
  - x/guide ship once, bf16, stacked as one [128, 4096] tensor per core,
  - all weights ship as a single packed f32 vector (~0.6 MB, built per call),
  - the channel-attention scalars (a 64-element sigmoid of the per-channel
    image mean) are computed on host,
  - output returns as bf16 [64, 4096] per core,
  - the jitted executable, mesh, and donated-zero placeholders are cached
    across calls (first call pays the NEFF compile).
On device each core runs the whole pipeline for its image: padded coord-conv
slab -> gated features -> q/k/v projections -> two 4096x4096 softmax
attentions (key-major, exp biased by an upper bound on the logits so no
transpose or running max is needed; the softmax denominator rides the AV
matmul as a ones-column of V^T) -> conv tail (c1/c2/sc).
"""
import sys
import numpy as np

sys.path.insert(0, "/opt/trn_rl_repo")

import concourse.bass as bass  # noqa: E402
import concourse.tile as tile  # noqa: E402
from concourse import bacc, mybir  # noqa: E402

F32 = mybir.dt.float32
FP16 = mybir.dt.float16
AF = mybir.ActivationFunctionType
ALU = mybir.AluOpType
AX = mybir.AxisListType

B, C, H, W = 2, 64, 64, 64
N = H * W              # 4096 pixels
PW = W + 2             # padded width/height 66
NPAD = PW * PW         # 4356 padded pixels
NT = N // 128          # 32 key tiles
NCH = N // 512         # 8 column chunks of 512

# wpack layout (f32 words)
_SEGS = [
    ("cw", 66 * 9 * C), ("c1w", C * 9 * C), ("c2w", C * 9 * C),
    ("wq", C * C), ("wk", C * C), ("wgq", C * C), ("wgk", C * C),
    ("scw", C * C), ("vtwb", 65 * C),
    ("bq", C), ("bk", C), ("bgq", C), ("bgk", C),
    ("c1b", C), ("c2b", C), ("scb", C),
    ("awx", C), ("awg", C), ("gam", 1), ("alpha", C),
    ("plate", 2 * NPAD),
]
_OFF = {}
_p = 0
for _nm, _sz in _SEGS:
    _OFF[_nm] = _p
    _p += _sz
WPACK = _p

_CACHE = {}


def _build_program():
    nc = bacc.Bacc(None, target_bir_lowering=False, debug=False, num_devices=2)

    xg_d = nc.dram_tensor("xg", [2 * C, N], FP16, kind="ExternalInput")
    wp_d = nc.dram_tensor("wpack", [WPACK], FP16, kind="ExternalInput")
    out_d = nc.dram_tensor("out", [C, N], FP16, kind="ExternalOutput")

    def wseg(name, p, c):
        o = _OFF[name]
        return wp_d[o:o + p * c].rearrange("(p c) -> p c", c=c)

    with tile.TileContext(nc) as tc:
        with (
            tc.tile_pool(name="const", bufs=1) as cp,
            tc.tile_pool(name="big", bufs=1) as bp,
            tc.tile_pool(name="small", bufs=2) as sp,
        ):
            # ---- load packed fp16 weights, widen to f32 in SBUF ----
            def wload(name, p, c):
                h = sp.tile([p, c], FP16, tag="wl_h")
                nc.sync.dma_start(h[:], wseg(name, p, c))
                t = cp.tile([p, c], F32, tag="w_" + name)
                nc.vector.tensor_copy(t[:], h[:])
                return t

            cw_s = wload("cw", 66, 9 * C)
            c1w_s = wload("c1w", C, 9 * C)
            c2w_s = wload("c2w", C, 9 * C)
            wq_s = wload("wq", C, C)
            wk_s = wload("wk", C, C)
            wgq_s = wload("wgq", C, C)
            wgk_s = wload("wgk", C, C)
            scw_s = wload("scw", C, C)
            vtwb_s = wload("vtwb", 65, C)
            bcol = {nm: wload(nm, C, 1)
                    for nm in ("bq", "bk", "bgq", "bgk", "c1b", "c2b", "scb",
                               "awx", "awg", "alpha")}
            gam_s = wload("gam", 1, 1)
            ones64 = cp.tile([C, 1], F32); nc.vector.memset(ones64[:], 1.0)

            # ---- inputs + padded coord slabs ----
            xg_s = bp.tile([2 * C, N], FP16, tag="xgbf")
            nc.sync.dma_start(xg_s[:], xg_d[:])

            cs_s = bp.tile([66, NPAD], F32, tag="slabA")
            gs_s = bp.tile([66, NPAD], F32, tag="slabB")
            nc.vector.memset(cs_s[0:C, :], 0.0)
            nc.vector.memset(gs_s[0:C, :], 0.0)
            plate_h = sp.tile([2, NPAD], FP16, tag="wl_plate")
            nc.sync.dma_start(plate_h[:], wseg("plate", 2, NPAD))
            nc.vector.tensor_copy(cs_s[C:66, :], plate_h[:])
            nc.vector.tensor_copy(gs_s[C:66, :], plate_h[:])
            cs3 = cs_s[:].rearrange("c (r w) -> c r w", w=PW)
            gs3 = gs_s[:].rearrange("c (r w) -> c r w", w=PW)
            xg3 = xg_s[:].rearrange("c (r w) -> c r w", w=W)
            nc.vector.tensor_copy(cs3[0:C, 1:1 + H, 1:1 + W], xg3[0:C])
            nc.vector.tensor_copy(gs3[0:C, 1:1 + H, 1:1 + W], xg3[C:2 * C])

            # ---- gated coord-conv features (row 64 = ones for bias folding) ----
            xgt = bp.tile([65, N], F32, tag="featA")
            ggt = bp.tile([65, N], F32, tag="featB")
            nc.vector.memset(xgt[64:65, :], 1.0)
            nc.vector.memset(ggt[64:65, :], 1.0)

            with tc.tile_pool(name="feps", bufs=3, space="PSUM") as fp:
                def coord_conv(slab3, aw, dst):
                    for g in range(8):
                        r0 = 8 * g
                        ps = fp.tile([C, 512], F32, tag="fe_ps")
                        for dy in range(3):
                            for dx in range(3):
                                nc.tensor.matmul(
                                    ps[:],
                                    cw_s[:, (dy * 3 + dx) * C:(dy * 3 + dx + 1) * C],
                                    slab3[:, r0 + dy:r0 + dy + 8, dx:dx + W],
                                    start=(dy == 0 and dx == 0),
                                    stop=(dy == 2 and dx == 2),
                                )
                        nc.vector.tensor_scalar_mul(
                            dst[0:C, r0 * W:(r0 + 8) * W], ps[:], aw[:, 0:1])

                coord_conv(cs3, bcol["awx"], xgt)
                coord_conv(gs3, bcol["awg"], ggt)

                # ---- 1x1 projections ----
                qx = bp.tile([C, N], F32, tag="projA")
                gqx = bp.tile([C, N], F32, tag="projB")
                kx = bp.tile([C, N], F32, tag="projC")
                gkx = bp.tile([C, N], F32, tag="projD")

                def lin(src, w_s, b_s, dst):
                    for g in range(NCH):
                        c0 = 512 * g
                        ps = fp.tile([C, 512], F32, tag="fe_ps")
                        nc.tensor.matmul(ps[:], w_s[:], src[0:C, c0:c0 + 512],
                                         start=True, stop=True)
                        nc.vector.tensor_scalar_add(dst[:, c0:c0 + 512], ps[:], b_s[:, 0:1])

                lin(xgt, wq_s, bcol["bq"], qx)
                lin(ggt, wgq_s, bcol["bgq"], gqx)
                lin(xgt, wk_s, bcol["bk"], kx)
                lin(ggt, wgk_s, bcol["bgk"], gkx)

                # V^T tiles [128 pixels, 65] (col 64 = ones for the row-sum)
                vtf = bp.tile([128, NT * 65], F32, tag="vt")
                vtf3 = vtf[:].rearrange("p (t e) -> p t e", e=65)
                nc.vector.memset(vtf[:], 1.0)
                for t in range(NT):
                    ps = fp.tile([128, C], F32, tag="fe_ps")
                    nc.tensor.matmul(ps[:], xgt[:, 128 * t:128 * (t + 1)],
                                     vtwb_s[:], start=True, stop=True)
                    nc.vector.tensor_copy(vtf3[:, t, 0:C], ps[:])

                # ---- max-norm stats -> exp biases ----
                sq = bp.tile([C, N], F32, tag="slabA")

                def sq_colmax(src, tagp):
                    nc.vector.tensor_mul(sq[:], src[0:C, :], src[0:C, :])
                    parts = sp.tile([1, NCH], F32, tag=tagp + "_p")
                    for g in range(NCH):
                        ps = fp.tile([1, 512], F32, tag="fe_ps")
                        nc.tensor.matmul(ps[:], ones64[:], sq[:, 512 * g:512 * (g + 1)],
                                         start=True, stop=True)
                        nc.vector.reduce_max(parts[:, g:g + 1], ps[0:1, :], axis=AX.X)
                    mx = sp.tile([1, 1], F32, tag=tagp)
                    nc.vector.reduce_max(mx[:], parts[0:1, :], axis=AX.X)
                    return mx

                k2x = sq_colmax(kx, "k2x")
                k2g = sq_colmax(gkx, "k2g")
                q2x = sq_colmax(qx, "q2x")
                q2g = sq_colmax(gqx, "q2g")

            def mk_bias(q2, k2, nm):
                t = sp.tile([1, 1], F32, tag="bias_t" + nm)
                nc.vector.tensor_add(t[:], q2[:], k2[:])
                nc.vector.tensor_scalar_mul(t[:], t[:], -0.5)
                col = cp.tile([128, 1], F32, tag="bias_col" + nm)
                nc.gpsimd.partition_broadcast(col[:], t[0:1, :])
                return col

            bias_x = mk_bias(q2x, k2x, "x")
            bias_g = mk_bias(q2g, k2g, "g")

            # ---- attention (guide first, then x; both use x's values) ----
            ong = bp.tile([C, N], F32, tag="featB")   # raw guide_out
            ocx = bp.tile([C, N], F32, tag="featA")   # gamma * x_out

            with (
                tc.tile_pool(name="aps_s", bufs=2, space="PSUM") as pss,
                tc.tile_pool(name="aps_o", bufs=2, space="PSUM") as pso,
                tc.tile_pool(name="atp", bufs=3) as atp,
            ):
                for (q_t, k_t, bias_c, dst, gscale) in (
                    (gqx, gkx, bias_g, ong, None),
                    (qx, kx, bias_x, ocx, gam_s),
                ):
                    for h in range(NCH):
                        o = pso.tile([65, 512], F32, tag="o_ps")
                        for t in range(NT):
                            s = pss.tile([128, 512], F32, tag="s_ps")
                            nc.tensor.matmul(s[:], k_t[:, 128 * t:128 * (t + 1)],
                                             q_t[:, 512 * h:512 * (h + 1)],
                                             start=True, stop=True)
                            at = atp.tile([128, 512], F32, tag="at")
                            nc.scalar.activation(at[:], s[:], AF.Exp,
                                                 bias=bias_c[:, 0:1], scale=1.0)
                            nc.tensor.matmul(o[:], vtf3[:, t, :], at[:],
                                             start=(t == 0), stop=(t == NT - 1))
                        rc = sp.tile([1, 512], F32, tag="rc")
                        nc.vector.reciprocal(rc[:], o[64:65, :])
                        if gscale is not None:
                            nc.vector.tensor_scalar_mul(rc[:], rc[:], gscale[0:1, 0:1])
                        rb = sp.tile([C, 512], F32, tag="rb")
                        nc.gpsimd.partition_broadcast(rb[:], rc[0:1, :])
                        nc.vector.tensor_mul(dst[:, 512 * h:512 * (h + 1)], o[0:C, :], rb[:])

            # ---- combine + conv tail ----
            oc = bp.tile([C, N], F32, tag="projA")
            tmpn = bp.tile([C, N], F32, tag="projC")
            nc.vector.tensor_scalar_mul(tmpn[:], ong[:], bcol["alpha"][:, 0:1])
            nc.vector.tensor_add(oc[:], ocx[:], tmpn[:])

            lks = bp.tile([C, NPAD], F32, tag="slabA")
            nc.vector.memset(lks[:], 0.0)
            lks3 = lks[:].rearrange("c (r w) -> c r w", w=PW)
            oc3 = oc[:].rearrange("c (r w) -> c r w", w=W)
            nc.vector.tensor_scalar_mul(tmpn[:], oc[:], 0.1)
            nc.vector.tensor_max(lks3[:, 1:1 + H, 1:1 + W], oc3[:],
                                 tmpn[:].rearrange("c (r w) -> c r w", w=W))

            c1s = bp.tile([C, NPAD], F32, tag="slabB")
            nc.vector.memset(c1s[:], 0.0)
            c1s3 = c1s[:].rearrange("c (r w) -> c r w", w=PW)

            branch = bp.tile([C, N], F32, tag="projB")
            finalv = bp.tile([C, N], F32, tag="projC")
            out_bf = bp.tile([C, N], FP16, tag="projD")

            with tc.tile_pool(name="beps", bufs=3, space="PSUM") as bps:
                def conv3(src3, w_s, g):
                    ps = bps.tile([C, 512], F32, tag="be_ps")
                    for dy in range(3):
                        for dx in range(3):
                            nc.tensor.matmul(
                                ps[:],
                                w_s[:, (dy * 3 + dx) * C:(dy * 3 + dx + 1) * C],
                                src3[:, 8 * g + dy:8 * g + dy + 8, dx:dx + W],
                                start=(dy == 0 and dx == 0), stop=(dy == 2 and dx == 2))
                    return ps

                # c1 + leaky -> padded slab
                for g in range(8):
                    ps = conv3(lks3, c1w_s, g)
                    tmp = sp.tile([C, 512], F32, tag="c1_tmp")
                    nc.vector.tensor_scalar_add(tmp[:], ps[:], bcol["c1b"][:, 0:1])
                    tmp2 = sp.tile([C, 512], F32, tag="c1_tmp2")
                    nc.vector.tensor_scalar_mul(tmp2[:], tmp[:], 0.1)
                    nc.vector.tensor_max(
                        c1s3[:, 8 * g + 1:8 * g + 9, 1:1 + W],
                        tmp[:].rearrange("c (r w) -> c r w", w=W),
                        tmp2[:].rearrange("c (r w) -> c r w", w=W))

                # c2 -> branch
                for g in range(8):
                    ps = conv3(c1s3, c2w_s, g)
                    nc.vector.tensor_scalar_add(branch[:, 512 * g:512 * (g + 1)],
                                                ps[:], bcol["c2b"][:, 0:1])

                # sc 1x1, final = branch + sc(oc) * guide_out
                for g in range(NCH):
                    c0 = 512 * g
                    ps = bps.tile([C, 512], F32, tag="be_ps")
                    nc.tensor.matmul(ps[:], scw_s[:], oc[:, c0:c0 + 512],
                                     start=True, stop=True)
                    tmp = sp.tile([C, 512], F32, tag="sc_tmp")
                    nc.vector.tensor_scalar_add(tmp[:], ps[:], bcol["scb"][:, 0:1])
                    nc.vector.tensor_mul(tmp[:], tmp[:], ong[:, c0:c0 + 512])
                    nc.vector.tensor_add(finalv[:, c0:c0 + 512], branch[:, c0:c0 + 512], tmp[:])

                nc.vector.tensor_copy(out_bf[:], finalv[:])
                nc.sync.dma_start(out_d[:], out_bf[:])

    nc.compile()
    return nc


def _coordplate():
    xx = (np.arange(W, dtype=np.float32) / (W - 1)) * 2 - 1
    yy = (np.arange(H, dtype=np.float32) / (H - 1)) * 2 - 1
    plate = np.zeros((2, PW, PW), np.float32)
    plate[0, 1:1 + H, 1:1 + W] = xx[None, :]
    plate[1, 1:1 + H, 1:1 + W] = yy[:, None]
    return plate.reshape(2 * NPAD)


def _taps(w):  # (O, I, 3, 3) -> [I, 9*O] tap-major
    o, i = w.shape[0], w.shape[1]
    out = np.empty((i, 9 * o), np.float32)
    for dy in range(3):
        for dx in range(3):
            out[:, (dy * 3 + dx) * o:(dy * 3 + dx + 1) * o] = w[:, :, dy, dx].T
    return out


def _host_inputs(inputs):
    """Build the concatenated per-core inputs: xg [2*128, N] bf16, wpack [2*WPACK] f32."""
    f = lambda k: np.asarray(inputs[k], np.float32)
    x, guide = f("x"), f("guide")
    lin_w, lin_b = float(f("lin_w")), float(f("lin_b"))
    gamma = float(f("gamma").reshape(-1)[0])
    alpha = float(f("alpha").reshape(-1)[0])

    xg = np.empty((2 * 2 * C, N), np.float16)
    for b in range(B):
        xg[2 * C * b:2 * C * b + C] = x[b].reshape(C, N)
        xg[2 * C * b + C:2 * C * (b + 1)] = guide[b].reshape(C, N)

    # channel attention on host: sigmoid(lw*leaky(lw*mean+lb)+lb), per batch
    def aw_of(a):  # (B,C,H,W) -> (B,C)
        p = a.mean(axis=(2, 3), dtype=np.float32) * lin_w + lin_b
        hh = np.where(p > 0, p, np.float32(0.1) * p)
        t = hh * lin_w + lin_b
        return (1.0 / (1.0 + np.exp(-t))).astype(np.float32)

    awx, awg = aw_of(x), aw_of(guide)

    vtwb = np.empty((65, C), np.float32)
    vtwb[0:C] = f("xv_w").T
    vtwb[C] = f("xv_b")

    wp = np.empty(WPACK, np.float16)

    def put(nm, val):
        o = _OFF[nm]
        wp[o:o + val.size] = val.ravel()

    put("cw", _taps(f("coord_w")))
    put("c1w", _taps(f("c1_w"))); put("c2w", _taps(f("c2_w")))
    put("wq", np.ascontiguousarray(f("xq_w").T)); put("bq", f("xq_b"))
    put("wk", np.ascontiguousarray(f("xk_w").T)); put("bk", f("xk_b"))
    put("wgq", np.ascontiguousarray(f("gq_w").T)); put("bgq", f("gq_b"))
    put("wgk", np.ascontiguousarray(f("gk_w").T)); put("bgk", f("gk_b"))
    put("scw", np.ascontiguousarray(f("sc_w").T)); put("scb", f("sc_b"))
    put("vtwb", vtwb)
    put("c1b", f("c1_b")); put("c2b", f("c2_b"))
    put("gam", np.float32(gamma)); put("alpha", np.full(C, alpha, np.float32))
    put("plate", _CACHE.setdefault("plate", _coordplate()))

    wpc = np.concatenate([wp, wp])
    for b in range(B):
        wpc[b * WPACK + _OFF["awx"]:b * WPACK + _OFF["awx"] + C] = awx[b]
        wpc[b * WPACK + _OFF["awg"]:b * WPACK + _OFF["awg"] + C] = awg[b]
    return xg, wpc


def _setup():
    import jax
    from jax.sharding import Mesh, PartitionSpec, NamedSharding
    from jax.experimental.shard_map import shard_map
    import concourse.bass2jax as bass2jax

    nc = _build_program()
    bass2jax.install_neuronx_cc_hook()

    partition_name = nc.partition_id_tensor.name if nc.partition_id_tensor else None
    in_names, out_names, out_avals = [], [], []
    for alloc in nc.m.functions[0].allocations:
        if not isinstance(alloc, mybir.MemoryLocationSet):
            continue
        name = alloc.memorylocations[0].name
        if alloc.kind == "ExternalInput":
            if name != partition_name:
                in_names.append(name)
        elif alloc.kind == "ExternalOutput":
            out_names.append(name)
            out_avals.append(jax.core.ShapedArray(
                tuple(alloc.tensor_shape), mybir.dt.np(alloc.dtype)))
    n_params = len(in_names)
    n_outs = len(out_avals)
    in_names_all = list(in_names) + out_names + ([partition_name] if partition_name else [])

    def _body(*args):
        operands = list(args)
        if partition_name is not None:
            operands.append(bass2jax.partition_id_tensor())
        outs = bass2jax._bass_exec_p.bind(
            *operands,
            out_avals=tuple(out_avals), in_names=tuple(in_names_all),
            out_names=tuple(out_names), lowering_input_output_aliases=(),
            sim_require_finite=True, sim_require_nnan=True, nc=nc)
        return tuple(outs)

    devices = jax.devices()[:2]
    mesh = Mesh(np.asarray(devices), ("core",))
    sharding = NamedSharding(mesh, PartitionSpec("core"))
    donate = tuple(range(n_params, n_params + n_outs))
    sharded = jax.jit(
        shard_map(_body, mesh=mesh,
                  in_specs=(PartitionSpec("core"),) * (n_params + n_outs),
                  out_specs=(PartitionSpec("core"),) * n_outs,
                  check_rep=False),
        donate_argnums=donate, keep_unused=True)

    zero_shapes = [(2 * a.shape[0], *a.shape[1:]) for a in out_avals]
    zero_dtypes = [a.dtype for a in out_avals]
    zfn = jax.jit(
        lambda: tuple(jax.numpy.zeros(s, d) for s, d in zip(zero_shapes, zero_dtypes)),
        out_shardings=tuple(sharding for _ in out_avals))

    st = {"nc": nc, "in_names": in_names, "sharded": sharded, "zfn": zfn,
          "sharding": sharding}
    return st


def kernel(**inputs):
    import jax
    st = _CACHE.get("st")
    if st is None:
        st = _CACHE["st"] = _setup()

    xg, wpc = _host_inputs(inputs)
    by_name = {"xg": xg, "wpack": wpc}
    args = [by_name[n] for n in st["in_names"]]
    zeros = st["zfn"]()
    outs = st["sharded"](*args, *zeros)
    res = np.asarray(jax.device_get(outs[0]))  # [2*C, N] fp16
    return res.astype(np.float32).reshape(B, C, H, W)


# revision 5
# speedup vs baseline: 1.1941x; 1.1941x over previous
"""Trainium2 Bass kernel for nn_Cooord_Attn (B=2,C=64,H=W=64, dual NxN attention).

Sharding: 2 cores, one batch image per core (attention is per-sample, so the
batch axis is embarrassingly parallel). The other 6 cores idle; at this size
the wall clock is dominated by host<->device transfer over the axon tunnel
(~14 MB/s), so the design minimizes wire bytes:
  - x/guide ship once, bf16, stacked as one [128, 4096] tensor per core,
  - all weights ship as a single packed f32 vector (~0.6 MB, built per call),
  - the channel-attention scalars (a 64-element sigmoid of the per-channel
    image mean) are computed on host,
  - output returns as bf16 [64, 4096] per core,
  - the jitted executable, mesh, and the zero output-placeholder buffers
    (the bass_exec custom call wants its outputs passed as parameters) are
    cached across calls; only the first call pays the NEFF compile.
On device each core runs the whole pipeline for its image: padded coord-conv
slab -> gated features -> q/k/v projections -> two 4096x4096 softmax
attentions (key-major, exp biased by an upper bound on the logits so no
transpose or running max is needed; the softmax denominator rides the AV
matmul as a ones-column of V^T) -> conv tail (c1/c2/sc).
"""
import sys
import numpy as np

sys.path.insert(0, "/opt/trn_rl_repo")

import concourse.bass as bass  # noqa: E402
import concourse.tile as tile  # noqa: E402
from concourse import bacc, mybir  # noqa: E402

F32 = mybir.dt.float32
FP16 = mybir.dt.float16
AF = mybir.ActivationFunctionType
ALU = mybir.AluOpType
AX = mybir.AxisListType

B, C, H, W = 2, 64, 64, 64
N = H * W              # 4096 pixels
PW = W + 2             # padded width/height 66
NPAD = PW * PW         # 4356 padded pixels
NT = N // 128          # 32 key tiles
NCH = N // 512         # 8 column chunks of 512

# wpack layout (f32 words)
_SEGS = [
    ("cw", 66 * 9 * C), ("c1w", C * 9 * C), ("c2w", C * 9 * C),
    ("wq", C * C), ("wk", C * C), ("wgq", C * C), ("wgk", C * C),
    ("scw", C * C), ("vtwb", 65 * C),
    ("bq", C), ("bk", C), ("bgq", C), ("bgk", C),
    ("c1b", C), ("c2b", C), ("scb", C),
    ("awx", C), ("awg", C), ("gam", 1), ("alpha", C),
    ("plate", 2 * NPAD),
]
_OFF = {}
_p = 0
for _nm, _sz in _SEGS:
    _OFF[_nm] = _p
    _p += _sz
WPACK = _p

_CACHE = {}


def _build_program():
    nc = bacc.Bacc(None, target_bir_lowering=False, debug=False, num_devices=2)

    xg_d = nc.dram_tensor("xg", [2 * C, N], FP16, kind="ExternalInput")
    wp_d = nc.dram_tensor("wpack", [WPACK], FP16, kind="ExternalInput")
    out_d = nc.dram_tensor("out", [C, N], FP16, kind="ExternalOutput")

    def wseg(name, p, c):
        o = _OFF[name]
        return wp_d[o:o + p * c].rearrange("(p c) -> p c", c=c)

    with tile.TileContext(nc) as tc:
        with (
            tc.tile_pool(name="const", bufs=1) as cp,
            tc.tile_pool(name="big", bufs=1) as bp,
            tc.tile_pool(name="small", bufs=2) as sp,
        ):
            # ---- load packed fp16 weights, widen to f32 in SBUF ----
            def wload(name, p, c):
                h = sp.tile([p, c], FP16, tag="wl_h")
                nc.sync.dma_start(h[:], wseg(name, p, c))
                t = cp.tile([p, c], F32, tag="w_" + name)
                nc.vector.tensor_copy(t[:], h[:])
                return t

            cw_s = wload("cw", 66, 9 * C)
            c1w_s = wload("c1w", C, 9 * C)
            c2w_s = wload("c2w", C, 9 * C)
            wq_s = wload("wq", C, C)
            wk_s = wload("wk", C, C)
            wgq_s = wload("wgq", C, C)
            wgk_s = wload("wgk", C, C)
            scw_s = wload("scw", C, C)
            vtwb_s = wload("vtwb", 65, C)
            bcol = {nm: wload(nm, C, 1)
                    for nm in ("bq", "bk", "bgq", "bgk", "c1b", "c2b", "scb",
                               "awx", "awg", "alpha")}
            gam_s = wload("gam", 1, 1)
            ones64 = cp.tile([C, 1], F32); nc.vector.memset(ones64[:], 1.0)

            # ---- inputs + padded coord slabs ----
            xg_s = bp.tile([2 * C, N], FP16, tag="xgbf")
            nc.sync.dma_start(xg_s[:], xg_d[:])

            cs_s = bp.tile([66, NPAD], F32, tag="slabA")
            gs_s = bp.tile([66, NPAD], F32, tag="slabB")
            nc.vector.memset(cs_s[0:C, :], 0.0)
            nc.vector.memset(gs_s[0:C, :], 0.0)
            plate_h = sp.tile([2, NPAD], FP16, tag="wl_plate")
            nc.sync.dma_start(plate_h[:], wseg("plate", 2, NPAD))
            nc.vector.tensor_copy(cs_s[C:66, :], plate_h[:])
            nc.vector.tensor_copy(gs_s[C:66, :], plate_h[:])
            cs3 = cs_s[:].rearrange("c (r w) -> c r w", w=PW)
            gs3 = gs_s[:].rearrange("c (r w) -> c r w", w=PW)
            xg3 = xg_s[:].rearrange("c (r w) -> c r w", w=W)
            nc.vector.tensor_copy(cs3[0:C, 1:1 + H, 1:1 + W], xg3[0:C])
            nc.vector.tensor_copy(gs3[0:C, 1:1 + H, 1:1 + W], xg3[C:2 * C])

            # ---- gated coord-conv features (row 64 = ones for bias folding) ----
            xgt = bp.tile([65, N], F32, tag="featA")
            ggt = bp.tile([65, N], F32, tag="featB")
            nc.vector.memset(xgt[64:65, :], 1.0)
            nc.vector.memset(ggt[64:65, :], 1.0)

            with tc.tile_pool(name="feps", bufs=3, space="PSUM") as fp:
                def coord_conv(slab3, aw, dst):
                    for g in range(8):
                        r0 = 8 * g
                        ps = fp.tile([C, 512], F32, tag="fe_ps")
                        for dy in range(3):
                            for dx in range(3):
                                nc.tensor.matmul(
                                    ps[:],
                                    cw_s[:, (dy * 3 + dx) * C:(dy * 3 + dx + 1) * C],
                                    slab3[:, r0 + dy:r0 + dy + 8, dx:dx + W],
                                    start=(dy == 0 and dx == 0),
                                    stop=(dy == 2 and dx == 2),
                                )
                        nc.vector.tensor_scalar_mul(
                            dst[0:C, r0 * W:(r0 + 8) * W], ps[:], aw[:, 0:1])

                coord_conv(cs3, bcol["awx"], xgt)
                coord_conv(gs3, bcol["awg"], ggt)

                # ---- 1x1 projections ----
                qx = bp.tile([C, N], F32, tag="projA")
                gqx = bp.tile([C, N], F32, tag="projB")
                kx = bp.tile([C, N], F32, tag="projC")
                gkx = bp.tile([C, N], F32, tag="projD")

                def lin(src, w_s, b_s, dst):
                    for g in range(NCH):
                        c0 = 512 * g
                        ps = fp.tile([C, 512], F32, tag="fe_ps")
                        nc.tensor.matmul(ps[:], w_s[:], src[0:C, c0:c0 + 512],
                                         start=True, stop=True)
                        nc.vector.tensor_scalar_add(dst[:, c0:c0 + 512], ps[:], b_s[:, 0:1])

                lin(xgt, wq_s, bcol["bq"], qx)
                lin(ggt, wgq_s, bcol["bgq"], gqx)
                lin(xgt, wk_s, bcol["bk"], kx)
                lin(ggt, wgk_s, bcol["bgk"], gkx)

                # V^T tiles [128 pixels, 65] (col 64 = ones for the row-sum)
                vtf = bp.tile([128, NT * 65], F32, tag="vt")
                vtf3 = vtf[:].rearrange("p (t e) -> p t e", e=65)
                nc.vector.memset(vtf[:], 1.0)
                for t in range(NT):
                    ps = fp.tile([128, C], F32, tag="fe_ps")
                    nc.tensor.matmul(ps[:], xgt[:, 128 * t:128 * (t + 1)],
                                     vtwb_s[:], start=True, stop=True)
                    nc.vector.tensor_copy(vtf3[:, t, 0:C], ps[:])

                # ---- max-norm stats -> exp biases ----
                sq = bp.tile([C, N], F32, tag="slabA")

                def sq_colmax(src, tagp):
                    nc.vector.tensor_mul(sq[:], src[0:C, :], src[0:C, :])
                    parts = sp.tile([1, NCH], F32, tag=tagp + "_p")
                    for g in range(NCH):
                        ps = fp.tile([1, 512], F32, tag="fe_ps")
                        nc.tensor.matmul(ps[:], ones64[:], sq[:, 512 * g:512 * (g + 1)],
                                         start=True, stop=True)
                        nc.vector.reduce_max(parts[:, g:g + 1], ps[0:1, :], axis=AX.X)
                    mx = sp.tile([1, 1], F32, tag=tagp)
                    nc.vector.reduce_max(mx[:], parts[0:1, :], axis=AX.X)
                    return mx

                k2x = sq_colmax(kx, "k2x")
                k2g = sq_colmax(gkx, "k2g")
                q2x = sq_colmax(qx, "q2x")
                q2g = sq_colmax(gqx, "q2g")

            def mk_bias(q2, k2, nm):
                t = sp.tile([1, 1], F32, tag="bias_t" + nm)
                nc.vector.tensor_add(t[:], q2[:], k2[:])
                nc.vector.tensor_scalar_mul(t[:], t[:], -0.5)
                col = cp.tile([128, 1], F32, tag="bias_col" + nm)
                nc.gpsimd.partition_broadcast(col[:], t[0:1, :])
                return col

            bias_x = mk_bias(q2x, k2x, "x")
            bias_g = mk_bias(q2g, k2g, "g")

            # ---- attention (guide first, then x; both use x's values) ----
            ong = bp.tile([C, N], F32, tag="featB")   # raw guide_out
            ocx = bp.tile([C, N], F32, tag="featA")   # gamma * x_out

            with (
                tc.tile_pool(name="aps_s", bufs=2, space="PSUM") as pss,
                tc.tile_pool(name="aps_o", bufs=2, space="PSUM") as pso,
                tc.tile_pool(name="atp", bufs=3) as atp,
            ):
                for (q_t, k_t, bias_c, dst, gscale) in (
                    (gqx, gkx, bias_g, ong, None),
                    (qx, kx, bias_x, ocx, gam_s),
                ):
                    for h in range(NCH):
                        o = pso.tile([65, 512], F32, tag="o_ps")
                        for t in range(NT):
                            s = pss.tile([128, 512], F32, tag="s_ps")
                            nc.tensor.matmul(s[:], k_t[:, 128 * t:128 * (t + 1)],
                                             q_t[:, 512 * h:512 * (h + 1)],
                                             start=True, stop=True)
                            at = atp.tile([128, 512], F32, tag="at")
                            nc.scalar.activation(at[:], s[:], AF.Exp,
                                                 bias=bias_c[:, 0:1], scale=1.0)
                            nc.tensor.matmul(o[:], vtf3[:, t, :], at[:],
                                             start=(t == 0), stop=(t == NT - 1))
                        rc = sp.tile([1, 512], F32, tag="rc")
                        nc.vector.reciprocal(rc[:], o[64:65, :])
                        if gscale is not None:
                            nc.vector.tensor_scalar_mul(rc[:], rc[:], gscale[0:1, 0:1])
                        rb = sp.tile([C, 512], F32, tag="rb")
                        nc.gpsimd.partition_broadcast(rb[:], rc[0:1, :])
                        nc.vector.tensor_mul(dst[:, 512 * h:512 * (h + 1)], o[0:C, :], rb[:])

            # ---- combine + conv tail ----
            oc = bp.tile([C, N], F32, tag="projA")
            tmpn = bp.tile([C, N], F32, tag="projC")
            nc.vector.tensor_scalar_mul(tmpn[:], ong[:], bcol["alpha"][:, 0:1])
            nc.vector.tensor_add(oc[:], ocx[:], tmpn[:])

            lks = bp.tile([C, NPAD], F32, tag="slabA")
            nc.vector.memset(lks[:], 0.0)
            lks3 = lks[:].rearrange("c (r w) -> c r w", w=PW)
            oc3 = oc[:].rearrange("c (r w) -> c r w", w=W)
            nc.vector.tensor_scalar_mul(tmpn[:], oc[:], 0.1)
            nc.vector.tensor_max(lks3[:, 1:1 + H, 1:1 + W], oc3[:],
                                 tmpn[:].rearrange("c (r w) -> c r w", w=W))

            c1s = bp.tile([C, NPAD], F32, tag="slabB")
            nc.vector.memset(c1s[:], 0.0)
            c1s3 = c1s[:].rearrange("c (r w) -> c r w", w=PW)

            branch = bp.tile([C, N], F32, tag="projB")
            finalv = bp.tile([C, N], F32, tag="projC")
            out_bf = bp.tile([C, N], FP16, tag="projD")

            with tc.tile_pool(name="beps", bufs=3, space="PSUM") as bps:
                def conv3(src3, w_s, g):
                    ps = bps.tile([C, 512], F32, tag="be_ps")
                    for dy in range(3):
                        for dx in range(3):
                            nc.tensor.matmul(
                                ps[:],
                                w_s[:, (dy * 3 + dx) * C:(dy * 3 + dx + 1) * C],
                                src3[:, 8 * g + dy:8 * g + dy + 8, dx:dx + W],
                                start=(dy == 0 and dx == 0), stop=(dy == 2 and dx == 2))
                    return ps

                # c1 + leaky -> padded slab
                for g in range(8):
                    ps = conv3(lks3, c1w_s, g)
                    tmp = sp.tile([C, 512], F32, tag="c1_tmp")
                    nc.vector.tensor_scalar_add(tmp[:], ps[:], bcol["c1b"][:, 0:1])
                    tmp2 = sp.tile([C, 512], F32, tag="c1_tmp2")
                    nc.vector.tensor_scalar_mul(tmp2[:], tmp[:], 0.1)
                    nc.vector.tensor_max(
                        c1s3[:, 8 * g + 1:8 * g + 9, 1:1 + W],
                        tmp[:].rearrange("c (r w) -> c r w", w=W),
                        tmp2[:].rearrange("c (r w) -> c r w", w=W))

                # c2 -> branch
                for g in range(8):
                    ps = conv3(c1s3, c2w_s, g)
                    nc.vector.tensor_scalar_add(branch[:, 512 * g:512 * (g + 1)],
                                                ps[:], bcol["c2b"][:, 0:1])

                # sc 1x1, final = branch + sc(oc) * guide_out
                for g in range(NCH):
                    c0 = 512 * g
                    ps = bps.tile([C, 512], F32, tag="be_ps")
                    nc.tensor.matmul(ps[:], scw_s[:], oc[:, c0:c0 + 512],
                                     start=True, stop=True)
                    tmp = sp.tile([C, 512], F32, tag="sc_tmp")
                    nc.vector.tensor_scalar_add(tmp[:], ps[:], bcol["scb"][:, 0:1])
                    nc.vector.tensor_mul(tmp[:], tmp[:], ong[:, c0:c0 + 512])
                    nc.vector.tensor_add(finalv[:, c0:c0 + 512], branch[:, c0:c0 + 512], tmp[:])

                nc.vector.tensor_copy(out_bf[:], finalv[:])
                nc.sync.dma_start(out_d[:], out_bf[:])

    nc.compile()
    return nc


def _coordplate():
    xx = (np.arange(W, dtype=np.float32) / (W - 1)) * 2 - 1
    yy = (np.arange(H, dtype=np.float32) / (H - 1)) * 2 - 1
    plate = np.zeros((2, PW, PW), np.float32)
    plate[0, 1:1 + H, 1:1 + W] = xx[None, :]
    plate[1, 1:1 + H, 1:1 + W] = yy[:, None]
    return plate.reshape(2 * NPAD)


def _taps(w):  # (O, I, 3, 3) -> [I, 9*O] tap-major
    o, i = w.shape[0], w.shape[1]
    out = np.empty((i, 9 * o), np.float32)
    for dy in range(3):
        for dx in range(3):
            out[:, (dy * 3 + dx) * o:(dy * 3 + dx + 1) * o] = w[:, :, dy, dx].T
    return out


def _host_inputs(inputs):
    """Build the concatenated per-core inputs: xg [2*128, N] bf16, wpack [2*WPACK] f32."""
    f = lambda k: np.asarray(inputs[k], np.float32)
    x, guide = f("x"), f("guide")
    lin_w, lin_b = float(f("lin_w")), float(f("lin_b"))
    gamma = float(f("gamma").reshape(-1)[0])
    alpha = float(f("alpha").reshape(-1)[0])

    xg = np.empty((2 * 2 * C, N), np.float16)
    for b in range(B):
        xg[2 * C * b:2 * C * b + C] = x[b].reshape(C, N)
        xg[2 * C * b + C:2 * C * (b + 1)] = guide[b].reshape(C, N)

    # channel attention on host: sigmoid(lw*leaky(lw*mean+lb)+lb), per batch
    def aw_of(a):  # (B,C,H,W) -> (B,C)
        p = a.mean(axis=(2, 3), dtype=np.float32) * lin_w + lin_b
        hh = np.where(p > 0, p, np.float32(0.1) * p)
        t = hh * lin_w + lin_b
        return (1.0 / (1.0 + np.exp(-t))).astype(np.float32)

    awx, awg = aw_of(x), aw_of(guide)

    vtwb = np.empty((65, C), np.float32)
    vtwb[0:C] = f("xv_w").T
    vtwb[C] = f("xv_b")

    wp = np.empty(WPACK, np.float16)

    def put(nm, val):
        o = _OFF[nm]
        wp[o:o + val.size] = val.ravel()

    put("cw", _taps(f("coord_w")))
    put("c1w", _taps(f("c1_w"))); put("c2w", _taps(f("c2_w")))
    put("wq", np.ascontiguousarray(f("xq_w").T)); put("bq", f("xq_b"))
    put("wk", np.ascontiguousarray(f("xk_w").T)); put("bk", f("xk_b"))
    put("wgq", np.ascontiguousarray(f("gq_w").T)); put("bgq", f("gq_b"))
    put("wgk", np.ascontiguousarray(f("gk_w").T)); put("bgk", f("gk_b"))
    put("scw", np.ascontiguousarray(f("sc_w").T)); put("scb", f("sc_b"))
    put("vtwb", vtwb)
    put("c1b", f("c1_b")); put("c2b", f("c2_b"))
    put("gam", np.float32(gamma)); put("alpha", np.full(C, alpha, np.float32))
    put("plate", _CACHE.setdefault("plate", _coordplate()))

    wpc = np.concatenate([wp, wp])
    for b in range(B):
        wpc[b * WPACK + _OFF["awx"]:b * WPACK + _OFF["awx"] + C] = awx[b]
        wpc[b * WPACK + _OFF["awg"]:b * WPACK + _OFF["awg"] + C] = awg[b]
    return xg, wpc


def _setup():
    import jax
    from jax.sharding import Mesh, PartitionSpec, NamedSharding
    from jax.experimental.shard_map import shard_map
    import concourse.bass2jax as bass2jax

    nc = _build_program()
    bass2jax.install_neuronx_cc_hook()

    partition_name = nc.partition_id_tensor.name if nc.partition_id_tensor else None
    in_names, out_names, out_avals = [], [], []
    for alloc in nc.m.functions[0].allocations:
        if not isinstance(alloc, mybir.MemoryLocationSet):
            continue
        name = alloc.memorylocations[0].name
        if alloc.kind == "ExternalInput":
            if name != partition_name:
                in_names.append(name)
        elif alloc.kind == "ExternalOutput":
            out_names.append(name)
            out_avals.append(jax.core.ShapedArray(
                tuple(alloc.tensor_shape), mybir.dt.np(alloc.dtype)))
    n_params = len(in_names)
    n_outs = len(out_avals)
    in_names_all = list(in_names) + out_names + ([partition_name] if partition_name else [])

    def _body(*args):
        operands = list(args)
        if partition_name is not None:
            operands.append(bass2jax.partition_id_tensor())
        outs = bass2jax._bass_exec_p.bind(
            *operands,
            out_avals=tuple(out_avals), in_names=tuple(in_names_all),
            out_names=tuple(out_names), lowering_input_output_aliases=(),
            sim_require_finite=True, sim_require_nnan=True, nc=nc)
        return tuple(outs)

    devices = jax.devices()[:2]
    mesh = Mesh(np.asarray(devices), ("core",))
    sharding = NamedSharding(mesh, PartitionSpec("core"))
    sharded = jax.jit(
        shard_map(_body, mesh=mesh,
                  in_specs=(PartitionSpec("core"),) * (n_params + n_outs),
                  out_specs=(PartitionSpec("core"),) * n_outs,
                  check_rep=False),
        keep_unused=True)

    # outputs are fully written by the kernel, so the placeholder buffers are
    # never read back: create them on device once and reuse every call
    zeros = tuple(
        jax.device_put(np.zeros((2 * a.shape[0], *a.shape[1:]), a.dtype), sharding)
        for a in out_avals)

    st = {"nc": nc, "in_names": in_names, "sharded": sharded, "zeros": zeros,
          "sharding": sharding}
    return st


def kernel(**inputs):
    import jax
    st = _CACHE.get("st")
    if st is None:
        st = _CACHE["st"] = _setup()

    xg, wpc = _host_inputs(inputs)
    by_name = {"xg": xg, "wpack": wpc}
    args = [by_name[n] for n in st["in_names"]]
    outs = st["sharded"](*args, *st["zeros"])
    res = np.asarray(jax.device_get(outs[0]))  # [2*C, N] fp16
    return res.astype(np.float32).reshape(B, C, H, W)


# revision 6
# speedup vs baseline: 1.4035x; 1.1754x over previous
"""Trainium2 Bass kernel for nn_Cooord_Attn (B=2,C=64,H=W=64, dual NxN attention).

Sharding: 2 cores, one batch image per core (attention is per-sample, so the
batch axis is embarrassingly parallel). The other 6 cores idle; at this size
the wall clock is dominated by host<->device transfer over the axon tunnel
(~14 MB/s), so the design minimizes wire bytes:
  - x/guide ship once, fp16, stacked as one [128, 4096] tensor per core,
  - all weights ship as a single packed fp16 vector (~0.3 MB, built per
    call), widened to f32 in SBUF,
  - the channel-attention scalars (a 64-element sigmoid of the per-channel
    image mean) are computed on host,
  - output returns as fp16 [64, 4096] per core,
  - the jitted executable, mesh, and the zero output-placeholder buffers
    (the bass_exec custom call wants its outputs passed as parameters) are
    cached across calls; only the first call pays the NEFF compile.
On device each core runs the whole pipeline for its image: padded coord-conv
slab -> gated features -> q/k/v projections -> two 4096x4096 softmax
attentions (key-major, exp biased by an upper bound on the logits so no
transpose or running max is needed; the softmax denominator rides the AV
matmul as a ones-column of V^T) -> conv tail (c1/c2/sc).
"""
import sys
import numpy as np

sys.path.insert(0, "/opt/trn_rl_repo")

import concourse.bass as bass  # noqa: E402
import concourse.tile as tile  # noqa: E402
from concourse import bacc, mybir  # noqa: E402

F32 = mybir.dt.float32
FP16 = mybir.dt.float16
AF = mybir.ActivationFunctionType
ALU = mybir.AluOpType
AX = mybir.AxisListType

B, C, H, W = 2, 64, 64, 64
N = H * W              # 4096 pixels
PW = W + 2             # padded width/height 66
NPAD = PW * PW         # 4356 padded pixels
NT = N // 128          # 32 key tiles
NCH = N // 512         # 8 column chunks of 512

# wpack layout (f32 words)
_SEGS = [
    ("cw", 66 * 9 * C), ("c1w", C * 9 * C), ("c2w", C * 9 * C),
    ("wq", C * C), ("wk", C * C), ("wgq", C * C), ("wgk", C * C),
    ("scw", C * C), ("vtwb", 65 * C),
    ("bq", C), ("bk", C), ("bgq", C), ("bgk", C),
    ("c1b", C), ("c2b", C), ("scb", C),
    ("awx", C), ("awg", C), ("gam", 1), ("alpha", C),
    ("plate", 2 * NPAD),
]
_OFF = {}
_p = 0
for _nm, _sz in _SEGS:
    _OFF[_nm] = _p
    _p += _sz
WPACK = _p

_CACHE = {}


def _build_program():
    nc = bacc.Bacc(None, target_bir_lowering=False, debug=False, num_devices=2)

    xg_d = nc.dram_tensor("xg", [2 * C, N], FP16, kind="ExternalInput")
    wp_d = nc.dram_tensor("wpack", [WPACK], FP16, kind="ExternalInput")
    out_d = nc.dram_tensor("out", [C, N], FP16, kind="ExternalOutput")

    def wseg(name, p, c):
        o = _OFF[name]
        return wp_d[o:o + p * c].rearrange("(p c) -> p c", c=c)

    with tile.TileContext(nc) as tc:
        with (
            tc.tile_pool(name="const", bufs=1) as cp,
            tc.tile_pool(name="big", bufs=1) as bp,
            tc.tile_pool(name="small", bufs=2) as sp,
        ):
            # ---- load packed fp16 weights, widen to f32 in SBUF ----
            def wload(name, p, c):
                h = sp.tile([p, c], FP16, tag="wl_h")
                nc.sync.dma_start(h[:], wseg(name, p, c))
                t = cp.tile([p, c], F32, tag="w_" + name)
                nc.vector.tensor_copy(t[:], h[:])
                return t

            cw_s = wload("cw", 66, 9 * C)
            c1w_s = wload("c1w", C, 9 * C)
            c2w_s = wload("c2w", C, 9 * C)
            wq_s = wload("wq", C, C)
            wk_s = wload("wk", C, C)
            wgq_s = wload("wgq", C, C)
            wgk_s = wload("wgk", C, C)
            scw_s = wload("scw", C, C)
            vtwb_s = wload("vtwb", 65, C)
            bcol = {nm: wload(nm, C, 1)
                    for nm in ("bq", "bk", "bgq", "bgk", "c1b", "c2b", "scb",
                               "awx", "awg", "alpha")}
            gam_s = wload("gam", 1, 1)
            ones64 = cp.tile([C, 1], F32); nc.vector.memset(ones64[:], 1.0)

            # ---- inputs + padded coord slabs ----
            xg_s = bp.tile([2 * C, N], FP16, tag="xgbf")
            nc.sync.dma_start(xg_s[:], xg_d[:])

            cs_s = bp.tile([66, NPAD], F32, tag="slabA")
            gs_s = bp.tile([66, NPAD], F32, tag="slabB")
            nc.vector.memset(cs_s[0:C, :], 0.0)
            nc.vector.memset(gs_s[0:C, :], 0.0)
            plate_h = sp.tile([2, NPAD], FP16, tag="wl_plate")
            nc.sync.dma_start(plate_h[:], wseg("plate", 2, NPAD))
            nc.vector.tensor_copy(cs_s[C:66, :], plate_h[:])
            nc.vector.tensor_copy(gs_s[C:66, :], plate_h[:])
            cs3 = cs_s[:].rearrange("c (r w) -> c r w", w=PW)
            gs3 = gs_s[:].rearrange("c (r w) -> c r w", w=PW)
            xg3 = xg_s[:].rearrange("c (r w) -> c r w", w=W)
            nc.vector.tensor_copy(cs3[0:C, 1:1 + H, 1:1 + W], xg3[0:C])
            nc.vector.tensor_copy(gs3[0:C, 1:1 + H, 1:1 + W], xg3[C:2 * C])

            # ---- gated coord-conv features (row 64 = ones for bias folding) ----
            xgt = bp.tile([65, N], F32, tag="featA")
            ggt = bp.tile([65, N], F32, tag="featB")
            nc.vector.memset(xgt[64:65, :], 1.0)
            nc.vector.memset(ggt[64:65, :], 1.0)

            with tc.tile_pool(name="feps", bufs=3, space="PSUM") as fp:
                def coord_conv(slab3, aw, dst):
                    for g in range(8):
                        r0 = 8 * g
                        ps = fp.tile([C, 512], F32, tag="fe_ps")
                        for dy in range(3):
                            for dx in range(3):
                                nc.tensor.matmul(
                                    ps[:],
                                    cw_s[:, (dy * 3 + dx) * C:(dy * 3 + dx + 1) * C],
                                    slab3[:, r0 + dy:r0 + dy + 8, dx:dx + W],
                                    start=(dy == 0 and dx == 0),
                                    stop=(dy == 2 and dx == 2),
                                )
                        nc.vector.tensor_scalar_mul(
                            dst[0:C, r0 * W:(r0 + 8) * W], ps[:], aw[:, 0:1])

                coord_conv(cs3, bcol["awx"], xgt)
                coord_conv(gs3, bcol["awg"], ggt)

                # ---- 1x1 projections ----
                qx = bp.tile([C, N], F32, tag="projA")
                gqx = bp.tile([C, N], F32, tag="projB")
                kx = bp.tile([C, N], F32, tag="projC")
                gkx = bp.tile([C, N], F32, tag="projD")

                def lin(src, w_s, b_s, dst):
                    for g in range(NCH):
                        c0 = 512 * g
                        ps = fp.tile([C, 512], F32, tag="fe_ps")
                        nc.tensor.matmul(ps[:], w_s[:], src[0:C, c0:c0 + 512],
                                         start=True, stop=True)
                        nc.vector.tensor_scalar_add(dst[:, c0:c0 + 512], ps[:], b_s[:, 0:1])

                lin(xgt, wq_s, bcol["bq"], qx)
                lin(ggt, wgq_s, bcol["bgq"], gqx)
                lin(xgt, wk_s, bcol["bk"], kx)
                lin(ggt, wgk_s, bcol["bgk"], gkx)

                # V^T tiles [128 pixels, 65] (col 64 = ones for the row-sum)
                vtf = bp.tile([128, NT * 65], F32, tag="vt")
                vtf3 = vtf[:].rearrange("p (t e) -> p t e", e=65)
                nc.vector.memset(vtf[:], 1.0)
                for t in range(NT):
                    ps = fp.tile([128, C], F32, tag="fe_ps")
                    nc.tensor.matmul(ps[:], xgt[:, 128 * t:128 * (t + 1)],
                                     vtwb_s[:], start=True, stop=True)
                    nc.vector.tensor_copy(vtf3[:, t, 0:C], ps[:])

                # ---- max-norm stats -> exp biases ----
                sq = bp.tile([C, N], F32, tag="slabA")

                def sq_colmax(src, tagp):
                    nc.vector.tensor_mul(sq[:], src[0:C, :], src[0:C, :])
                    parts = sp.tile([1, NCH], F32, tag=tagp + "_p")
                    for g in range(NCH):
                        ps = fp.tile([1, 512], F32, tag="fe_ps")
                        nc.tensor.matmul(ps[:], ones64[:], sq[:, 512 * g:512 * (g + 1)],
                                         start=True, stop=True)
                        nc.vector.reduce_max(parts[:, g:g + 1], ps[0:1, :], axis=AX.X)
                    mx = sp.tile([1, 1], F32, tag=tagp)
                    nc.vector.reduce_max(mx[:], parts[0:1, :], axis=AX.X)
                    return mx

                k2x = sq_colmax(kx, "k2x")
                k2g = sq_colmax(gkx, "k2g")
                q2x = sq_colmax(qx, "q2x")
                q2g = sq_colmax(gqx, "q2g")

            def mk_bias(q2, k2, nm):
                t = sp.tile([1, 1], F32, tag="bias_t" + nm)
                nc.vector.tensor_add(t[:], q2[:], k2[:])
                nc.vector.tensor_scalar_mul(t[:], t[:], -0.5)
                col = cp.tile([128, 1], F32, tag="bias_col" + nm)
                nc.gpsimd.partition_broadcast(col[:], t[0:1, :])
                return col

            bias_x = mk_bias(q2x, k2x, "x")
            bias_g = mk_bias(q2g, k2g, "g")

            # ---- attention (guide first, then x; both use x's values) ----
            ong = bp.tile([C, N], F32, tag="featB")   # raw guide_out
            ocx = bp.tile([C, N], F32, tag="featA")   # gamma * x_out

            with (
                tc.tile_pool(name="aps_s", bufs=2, space="PSUM") as pss,
                tc.tile_pool(name="aps_o", bufs=2, space="PSUM") as pso,
                tc.tile_pool(name="atp", bufs=3) as atp,
            ):
                for (q_t, k_t, bias_c, dst, gscale) in (
                    (gqx, gkx, bias_g, ong, None),
                    (qx, kx, bias_x, ocx, gam_s),
                ):
                    for h in range(NCH):
                        o = pso.tile([65, 512], F32, tag="o_ps")
                        for t in range(NT):
                            s = pss.tile([128, 512], F32, tag="s_ps")
                            nc.tensor.matmul(s[:], k_t[:, 128 * t:128 * (t + 1)],
                                             q_t[:, 512 * h:512 * (h + 1)],
                                             start=True, stop=True)
                            at = atp.tile([128, 512], F32, tag="at")
                            nc.scalar.activation(at[:], s[:], AF.Exp,
                                                 bias=bias_c[:, 0:1], scale=1.0)
                            nc.tensor.matmul(o[:], vtf3[:, t, :], at[:],
                                             start=(t == 0), stop=(t == NT - 1))
                        rc = sp.tile([1, 512], F32, tag="rc")
                        nc.vector.reciprocal(rc[:], o[64:65, :])
                        if gscale is not None:
                            nc.vector.tensor_scalar_mul(rc[:], rc[:], gscale[0:1, 0:1])
                        rb = sp.tile([C, 512], F32, tag="rb")
                        nc.gpsimd.partition_broadcast(rb[:], rc[0:1, :])
                        nc.vector.tensor_mul(dst[:, 512 * h:512 * (h + 1)], o[0:C, :], rb[:])

            # ---- combine + conv tail ----
            oc = bp.tile([C, N], F32, tag="projA")
            tmpn = bp.tile([C, N], F32, tag="projC")
            nc.vector.tensor_scalar_mul(tmpn[:], ong[:], bcol["alpha"][:, 0:1])
            nc.vector.tensor_add(oc[:], ocx[:], tmpn[:])

            lks = bp.tile([C, NPAD], F32, tag="slabA")
            nc.vector.memset(lks[:], 0.0)
            lks3 = lks[:].rearrange("c (r w) -> c r w", w=PW)
            oc3 = oc[:].rearrange("c (r w) -> c r w", w=W)
            nc.vector.tensor_scalar_mul(tmpn[:], oc[:], 0.1)
            nc.vector.tensor_max(lks3[:, 1:1 + H, 1:1 + W], oc3[:],
                                 tmpn[:].rearrange("c (r w) -> c r w", w=W))

            c1s = bp.tile([C, NPAD], F32, tag="slabB")
            nc.vector.memset(c1s[:], 0.0)
            c1s3 = c1s[:].rearrange("c (r w) -> c r w", w=PW)

            branch = bp.tile([C, N], F32, tag="projB")
            finalv = bp.tile([C, N], F32, tag="projC")
            out_bf = bp.tile([C, N], FP16, tag="projD")

            with tc.tile_pool(name="beps", bufs=3, space="PSUM") as bps:
                def conv3(src3, w_s, g):
                    ps = bps.tile([C, 512], F32, tag="be_ps")
                    for dy in range(3):
                        for dx in range(3):
                            nc.tensor.matmul(
                                ps[:],
                                w_s[:, (dy * 3 + dx) * C:(dy * 3 + dx + 1) * C],
                                src3[:, 8 * g + dy:8 * g + dy + 8, dx:dx + W],
                                start=(dy == 0 and dx == 0), stop=(dy == 2 and dx == 2))
                    return ps

                # c1 + leaky -> padded slab
                for g in range(8):
                    ps = conv3(lks3, c1w_s, g)
                    tmp = sp.tile([C, 512], F32, tag="c1_tmp")
                    nc.vector.tensor_scalar_add(tmp[:], ps[:], bcol["c1b"][:, 0:1])
                    tmp2 = sp.tile([C, 512], F32, tag="c1_tmp2")
                    nc.vector.tensor_scalar_mul(tmp2[:], tmp[:], 0.1)
                    nc.vector.tensor_max(
                        c1s3[:, 8 * g + 1:8 * g + 9, 1:1 + W],
                        tmp[:].rearrange("c (r w) -> c r w", w=W),
                        tmp2[:].rearrange("c (r w) -> c r w", w=W))

                # c2 -> branch
                for g in range(8):
                    ps = conv3(c1s3, c2w_s, g)
                    nc.vector.tensor_scalar_add(branch[:, 512 * g:512 * (g + 1)],
                                                ps[:], bcol["c2b"][:, 0:1])

                # sc 1x1, final = branch + sc(oc) * guide_out
                for g in range(NCH):
                    c0 = 512 * g
                    ps = bps.tile([C, 512], F32, tag="be_ps")
                    nc.tensor.matmul(ps[:], scw_s[:], oc[:, c0:c0 + 512],
                                     start=True, stop=True)
                    tmp = sp.tile([C, 512], F32, tag="sc_tmp")
                    nc.vector.tensor_scalar_add(tmp[:], ps[:], bcol["scb"][:, 0:1])
                    nc.vector.tensor_mul(tmp[:], tmp[:], ong[:, c0:c0 + 512])
                    nc.vector.tensor_add(finalv[:, c0:c0 + 512], branch[:, c0:c0 + 512], tmp[:])

                nc.vector.tensor_copy(out_bf[:], finalv[:])
                nc.sync.dma_start(out_d[:], out_bf[:])

    nc.compile()
    return nc


def _coordplate():
    xx = (np.arange(W, dtype=np.float32) / (W - 1)) * 2 - 1
    yy = (np.arange(H, dtype=np.float32) / (H - 1)) * 2 - 1
    plate = np.zeros((2, PW, PW), np.float32)
    plate[0, 1:1 + H, 1:1 + W] = xx[None, :]
    plate[1, 1:1 + H, 1:1 + W] = yy[:, None]
    return plate.reshape(2 * NPAD)


def _taps(w):  # (O, I, 3, 3) -> [I, 9*O] tap-major
    o, i = w.shape[0], w.shape[1]
    out = np.empty((i, 9 * o), np.float32)
    for dy in range(3):
        for dx in range(3):
            out[:, (dy * 3 + dx) * o:(dy * 3 + dx + 1) * o] = w[:, :, dy, dx].T
    return out


def _host_inputs(inputs):
    """Build the concatenated per-core inputs: xg [2*128, N] bf16, wpack [2*WPACK] f32."""
    f = lambda k: np.asarray(inputs[k], np.float32)
    x, guide = f("x"), f("guide")
    lin_w, lin_b = float(f("lin_w")), float(f("lin_b"))
    gamma = float(f("gamma").reshape(-1)[0])
    alpha = float(f("alpha").reshape(-1)[0])

    xg = np.empty((2 * 2 * C, N), np.float16)
    for b in range(B):
        xg[2 * C * b:2 * C * b + C] = x[b].reshape(C, N)
        xg[2 * C * b + C:2 * C * (b + 1)] = guide[b].reshape(C, N)

    # channel attention on host: sigmoid(lw*leaky(lw*mean+lb)+lb), per batch
    def aw_of(a):  # (B,C,H,W) -> (B,C)
        p = a.mean(axis=(2, 3), dtype=np.float32) * lin_w + lin_b
        hh = np.where(p > 0, p, np.float32(0.1) * p)
        t = hh * lin_w + lin_b
        return (1.0 / (1.0 + np.exp(-t))).astype(np.float32)

    awx, awg = aw_of(x), aw_of(guide)

    vtwb = np.empty((65, C), np.float32)
    vtwb[0:C] = f("xv_w").T
    vtwb[C] = f("xv_b")

    wp = np.empty(WPACK, np.float16)

    def put(nm, val):
        o = _OFF[nm]
        wp[o:o + val.size] = val.ravel()

    put("cw", _taps(f("coord_w")))
    put("c1w", _taps(f("c1_w"))); put("c2w", _taps(f("c2_w")))
    put("wq", np.ascontiguousarray(f("xq_w").T)); put("bq", f("xq_b"))
    put("wk", np.ascontiguousarray(f("xk_w").T)); put("bk", f("xk_b"))
    put("wgq", np.ascontiguousarray(f("gq_w").T)); put("bgq", f("gq_b"))
    put("wgk", np.ascontiguousarray(f("gk_w").T)); put("bgk", f("gk_b"))
    put("scw", np.ascontiguousarray(f("sc_w").T)); put("scb", f("sc_b"))
    put("vtwb", vtwb)
    put("c1b", f("c1_b")); put("c2b", f("c2_b"))
    put("gam", np.float32(gamma)); put("alpha", np.full(C, alpha, np.float32))
    put("plate", _CACHE.setdefault("plate", _coordplate()))

    wpc = np.concatenate([wp, wp])
    for b in range(B):
        wpc[b * WPACK + _OFF["awx"]:b * WPACK + _OFF["awx"] + C] = awx[b]
        wpc[b * WPACK + _OFF["awg"]:b * WPACK + _OFF["awg"] + C] = awg[b]
    return xg, wpc


def _setup():
    import jax
    from jax.sharding import Mesh, PartitionSpec, NamedSharding
    from jax.experimental.shard_map import shard_map
    import concourse.bass2jax as bass2jax

    nc = _build_program()
    bass2jax.install_neuronx_cc_hook()

    partition_name = nc.partition_id_tensor.name if nc.partition_id_tensor else None
    in_names, out_names, out_avals = [], [], []
    for alloc in nc.m.functions[0].allocations:
        if not isinstance(alloc, mybir.MemoryLocationSet):
            continue
        name = alloc.memorylocations[0].name
        if alloc.kind == "ExternalInput":
            if name != partition_name:
                in_names.append(name)
        elif alloc.kind == "ExternalOutput":
            out_names.append(name)
            out_avals.append(jax.core.ShapedArray(
                tuple(alloc.tensor_shape), mybir.dt.np(alloc.dtype)))
    n_params = len(in_names)
    n_outs = len(out_avals)
    in_names_all = list(in_names) + out_names + ([partition_name] if partition_name else [])

    def _body(*args):
        operands = list(args)
        if partition_name is not None:
            operands.append(bass2jax.partition_id_tensor())
        outs = bass2jax._bass_exec_p.bind(
            *operands,
            out_avals=tuple(out_avals), in_names=tuple(in_names_all),
            out_names=tuple(out_names), lowering_input_output_aliases=(),
            sim_require_finite=True, sim_require_nnan=True, nc=nc)
        return tuple(outs)

    devices = jax.devices()[:2]
    mesh = Mesh(np.asarray(devices), ("core",))
    sharding = NamedSharding(mesh, PartitionSpec("core"))
    sharded = jax.jit(
        shard_map(_body, mesh=mesh,
                  in_specs=(PartitionSpec("core"),) * (n_params + n_outs),
                  out_specs=(PartitionSpec("core"),) * n_outs,
                  check_rep=False),
        keep_unused=True)

    # outputs are fully written by the kernel, so the placeholder buffers are
    # never read back: create them on device once and reuse every call
    zeros = tuple(
        jax.device_put(np.zeros((2 * a.shape[0], *a.shape[1:]), a.dtype), sharding)
        for a in out_avals)

    st = {"nc": nc, "in_names": in_names, "sharded": sharded, "zeros": zeros,
          "sharding": sharding}
    return st


def kernel(**inputs):
    import jax
    st = _CACHE.get("st")
    if st is None:
        st = _CACHE["st"] = _setup()

    xg, wpc = _host_inputs(inputs)
    by_name = {"xg": xg, "wpack": wpc}
    args = [by_name[n] for n in st["in_names"]]
    outs = st["sharded"](*args, *st["zeros"])
    res = np.asarray(jax.device_get(outs[0]))  # [2*C, N] fp16
    return res.astype(np.float32).reshape(B, C, H, W)
